# revision 1
# baseline (speedup 1.0000x reference)
"""Trainium2 Bass kernel for the ELGCA block (dwconv3x3+gelu || conv1x1+gelu
-> pooled linear attention), data-parallel over batch on 8 NeuronCores.

Self-contained: hardcodes shapes B=16, C=128, H=W=128, f32.
kernel(**inputs) takes full unsharded inputs, returns full output.

Per-core layout (BPC=2 local images b0,b1):
  x1 path: partitions p=(b*64+c), dwconv via 9 per-partition bf16 MACs
           (dual shifted bf16 copies keep every tap 4B-aligned for 2x mode).
  conv1x1: one block-diagonal matmul per 512-col chunk per head-pair;
           A psum = [q(b0)|q(b1)|k(b0)|k(b1)], B = [v(b0)|v(b1)|l(b0)|l(b1)]
           so both batches share one gelu / one pool op per range.
"""

import numpy as np
from contextlib import ExitStack

import concourse.bass as bass
import concourse.tile as tile
from concourse import bacc, mybir
from concourse import bass_utils
from concourse.masks import make_identity

F32 = mybir.dt.float32
BF16 = mybir.dt.bfloat16
AX = mybir.AxisListType
ALU = mybir.AluOpType
ACTF = mybir.ActivationFunctionType

N_CORES = 8
B_TOT, C, H, W = 16, 128, 128, 128
BPC = B_TOT // N_CORES          # 2 images per core
HW = H * W                      # 16384
C2 = C // 2                     # 64
C4 = C // 4                     # 32
WP = W + 2                      # padded row width for dwconv
R = 16                          # dwconv row-strip height
NCH = 512                       # conv1x1 / out2 column chunk
NCHUNKS = HW // NCH             # 32
NP = (H // 2) * (W // 2)        # 4096 pooled positions
NTR = NP // 128                 # 32 transpose chunks

# dwconv taps in row-major (dy, dx) order
TAPS = [(dy, dx) for dy in (-1, 0, 1) for dx in (-1, 0, 1)]


def build_nc(loops=1):
    nc = bacc.Bacc("TRN2", target_bir_lowering=False, debug=False,
                   num_devices=N_CORES)
    x = nc.dram_tensor("x", [BPC, C, H, W], F32, kind="ExternalInput").ap()
    dw_w = nc.dram_tensor("dw_w", [C2, 1, 3, 3], F32, kind="ExternalInput").ap()
    dw_b = nc.dram_tensor("dw_b", [C2], F32, kind="ExternalInput").ap()
    qw = nc.dram_tensor("qkvl_w", [C, C2, 1, 1], F32, kind="ExternalInput").ap()
    qb = nc.dram_tensor("qkvl_b", [C], F32, kind="ExternalInput").ap()
    out = nc.dram_tensor("out", [BPC, C, H, W], F32, kind="ExternalOutput").ap()

    x2f = [x[b, C2:C, :, :].rearrange("c h w -> c (h w)") for b in range(BPC)]
    outf = [out[b].rearrange("c h w -> c (h w)") for b in range(BPC)]

    with tile.TileContext(nc) as tc, ExitStack() as ctx:
        consts = ctx.enter_context(tc.tile_pool(name="consts", bufs=1))
        x1p = ctx.enter_context(tc.tile_pool(name="x1p", bufs=2))
        rhsp = ctx.enter_context(tc.tile_pool(name="rhsp", bufs=2))
        packp = ctx.enter_context(tc.tile_pool(name="packp", bufs=1))
        lp = ctx.enter_context(tc.tile_pool(name="lp", bufs=1))
        poolt = ctx.enter_context(tc.tile_pool(name="poolt", bufs=1))
        attnp = ctx.enter_context(tc.tile_pool(name="attnp", bufs=1))
        stgp = ctx.enter_context(tc.tile_pool(name="stgp", bufs=1))
        ps_conv = ctx.enter_context(
            tc.tile_pool(name="ps_conv", bufs=2, space="PSUM"))
        ps_tr = ctx.enter_context(
            tc.tile_pool(name="ps_tr", bufs=1, space="PSUM"))
        ps_sm = ctx.enter_context(
            tc.tile_pool(name="ps_sm", bufs=2, space="PSUM"))
        ps_o2 = ctx.enter_context(
            tc.tile_pool(name="ps_o2", bufs=3, space="PSUM"))

        # ---------------- constants ----------------
        w_tile = consts.tile([128, 9], F32)          # dw weights per (b,c)
        dw9 = dw_w.rearrange("c o kh kw -> c (o kh kw)")
        nc.sync.dma_start(w_tile[0:C2, :], dw9)
        nc.sync.dma_start(w_tile[C2:128, :], dw9)

        dwb_t = consts.tile([128, 1], F32)
        dwb2 = dw_b.unsqueeze(1)
        nc.sync.dma_start(dwb_t[0:C2, :], dwb2)
        nc.sync.dma_start(dwb_t[C2:128, :], dwb2)

        # conv1x1 weights, block-diagonal with batch-contiguous head layout:
        # A out rows = [q(b0) 0:32 | q(b1) 32:64 | k(b0) 64:96 | k(b1) 96:128]
        # B out rows = [v(b0) 0:32 | v(b1) 32:64 | l(b0) 64:96 | l(b1) 96:128]
        qwT = qw.rearrange("o i kh kw -> (i kh kw) o")   # [64ic, 128oc] view
        lhsA = consts.tile([128, 128], F32)
        lhsB = consts.tile([128, 128], F32)
        nc.vector.memset(lhsA[:], 0.0)
        nc.vector.memset(lhsB[:], 0.0)
        nc.sync.dma_start(lhsA[0:C2, 0:C4], qwT[:, 0:C4])
        nc.sync.dma_start(lhsA[C2:128, C4:C2], qwT[:, 0:C4])
        nc.sync.dma_start(lhsA[0:C2, C2:96], qwT[:, C4:C2])
        nc.sync.dma_start(lhsA[C2:128, 96:128], qwT[:, C4:C2])
        nc.sync.dma_start(lhsB[0:C2, 0:C4], qwT[:, C2:96])
        nc.sync.dma_start(lhsB[C2:128, C4:C2], qwT[:, C2:96])
        nc.sync.dma_start(lhsB[0:C2, C2:96], qwT[:, 96:128])
        nc.sync.dma_start(lhsB[C2:128, 96:128], qwT[:, 96:128])

        qb2 = qb.unsqueeze(1)
        biasA = consts.tile([128, 1], F32)
        biasB = consts.tile([128, 1], F32)
        nc.sync.dma_start(biasA[0:C4, :], qb2[0:C4])
        nc.sync.dma_start(biasA[C4:C2, :], qb2[0:C4])
        nc.sync.dma_start(biasA[C2:96, :], qb2[C4:C2])
        nc.sync.dma_start(biasA[96:128, :], qb2[C4:C2])
        nc.sync.dma_start(biasB[0:C4, :], qb2[C2:96])
        nc.sync.dma_start(biasB[C4:C2, :], qb2[C2:96])
        nc.sync.dma_start(biasB[C2:96, :], qb2[96:128])
        nc.sync.dma_start(biasB[96:128, :], qb2[96:128])

        id_f32 = consts.tile([128, 128], F32)
        make_identity(nc, id_f32[:])

        RPC = NCH // W                    # image rows per conv chunk (4)

        def one_pass():
            # v_pack (bf16): [v(b0) 0:32 | v(b1) 32:64]
            v_pack = packp.tile([C2, HW], BF16, tag="v_pack")
            # rt (f32): h-pooled [q(b0)|q(b1) sums 0:64 | k(b0)|k(b1) max 64:128]
            rt = packp.tile([128, H * (W // 2)], F32, tag="rt")

            x1_strip_emitted = [0]

            def emit_x1_strip(s):
                y0 = s * R
                ys = max(y0 - 1, 0)
                ye = min(y0 + R + 1, H)
                rs = 0 if s > 0 else 1
                nrows = ye - ys

                xin = x1p.tile([128, (R + 2) * WP], F32, tag="xin")
                xin3 = xin.rearrange("p (r w) -> p r w", w=WP)
                nc.gpsimd.memset(xin3[:, :, 0:1], 0.0)
                nc.gpsimd.memset(xin3[:, :, WP - 1:WP], 0.0)
                if s == 0:
                    nc.gpsimd.memset(xin3[:, 0:1, :], 0.0)
                if s == H // R - 1:
                    nc.gpsimd.memset(xin3[:, R + 1:R + 2, :], 0.0)
                nc.scalar.dma_start(xin3[0:C2, rs:rs + nrows, 1:W + 1],
                                    x[0, 0:C2, ys:ye, :])
                nc.scalar.dma_start(xin3[C2:128, rs:rs + nrows, 1:W + 1],
                                    x[1, 0:C2, ys:ye, :])

                # bf16 cast + 1-elem-shifted bf16 copy, both on GpSimd
                # (no alignment-gated modes there). All 9 taps then read at
                # even element offsets (xb for dx=+-1, xs for dx=0) so each
                # DVE TS-mul runs 4x and each TT-add runs 2x.
                nflat = (R + 2) * WP
                xb = x1p.tile([128, nflat], BF16, tag="xb")
                nc.vector.tensor_copy(xb[:], xin[:])
                xs = x1p.tile([128, nflat], BF16, tag="xs")
                nc.gpsimd.tensor_copy(xs[:, 0:nflat - 2], xb[:, 1:nflat - 1])
                xb3 = xb.rearrange("p (r w) -> p r w", w=WP)
                xs3 = xs.rearrange("p (r w) -> p r w", w=WP)

                def tap_src(dy, dx):
                    if dx == 0:
                        return xs3[:, 1 + dy:1 + dy + R, 0:W]
                    elif dx == -1:
                        return xb3[:, 1 + dy:1 + dy + R, 0:W]
                    return xb3[:, 1 + dy:1 + dy + R, 2:2 + W]

                acc = x1p.tile([128, R * W], BF16, tag="acc")
                acc3 = acc.rearrange("p (r w) -> p r w", w=W)
                tap_order = ([t for t, (dy, dx) in enumerate(TAPS) if dx != 0]
                             + [t for t, (dy, dx) in enumerate(TAPS)
                                if dx == 0])
                for i, t in enumerate(tap_order):
                    dy, dx = TAPS[t]
                    src = tap_src(dy, dx)
                    wcol = w_tile[:, t:t + 1]
                    if i == 0:
                        nc.vector.tensor_scalar_mul(acc3[:], src, wcol)
                    else:
                        tmp = x1p.tile([128, R * W], BF16, tag="tmp", bufs=2)
                        tmp3 = tmp.rearrange("p (r w) -> p r w", w=W)
                        nc.vector.tensor_scalar_mul(tmp3[:], src, wcol)
                        nc.vector.tensor_add(acc[:], acc[:], tmp[:])

                ox1 = x1p.tile([128, R * W], F32, tag="ox1")
                nc.scalar.activation(ox1[:], acc[:], ACTF.Gelu,
                                     bias=dwb_t[:, 0:1])
                ox3 = ox1.rearrange("p (r w) -> p r w", w=W)
                nc.scalar.dma_start(out[0, 0:C2, y0:y0 + R, :], ox3[0:C2])
                nc.scalar.dma_start(out[1, 0:C2, y0:y0 + R, :], ox3[C2:128])

            # qf[0:64] = [qf(b0)|qf(b1)]; kf[64:128] = [kf(b0)|kf(b1)]
            qf = poolt.tile([C2, NP], F32, tag="qf")
            kf = poolt.tile([128, NP], F32, tag="kf")
            qfTs = [attnp.tile([128, NTR * C4], F32, tag=f"qfT{b}",
                               name=f"qfT{b}") for b in range(BPC)]
            kfTs = [attnp.tile([128, NTR * C4], F32, tag=f"kfT{b}",
                               name=f"kfT{b}") for b in range(BPC)]
            qkts_ps = [ps_sm.tile([C4, C4], F32, tag="sm", name=f"qkt{b}")
                       for b in range(BPC)]

            def emit_attn_half(h):
                # vertical pool for output rows oy in [h*32, (h+1)*32)
                oy0, oy1 = h * C4, (h + 1) * C4
                rq = rt[0:C2, :].rearrange(
                    "p (h2 two w2) -> p h2 two w2", two=2, w2=W // 2)
                qf3 = qf.rearrange("p (h2 w2) -> p h2 w2", w2=W // 2)
                nc.gpsimd.tensor_add(qf3[:, oy0:oy1, :],
                                     rq[:, oy0:oy1, 0, :],
                                     rq[:, oy0:oy1, 1, :])
                lo = max(oy0, 1)
                nc.gpsimd.tensor_add(qf3[:, lo:oy1, :], qf3[:, lo:oy1, :],
                                     rq[:, lo - 1:oy1 - 1, 1, :])
                rk = rt[C2:128, :].rearrange(
                    "p (h2 two w2) -> p h2 two w2", two=2, w2=W // 2)
                kf3 = kf[C2:128, :].rearrange("p (h2 w2) -> p h2 w2",
                                              w2=W // 2)
                nc.vector.tensor_max(kf3[:, oy0:oy1, :],
                                     rk[:, oy0:oy1, 0, :],
                                     rk[:, oy0:oy1, 1, :])
                # k(b1) at base 96 (illegal matmul base) -> kf[32:64]
                hcols = bass.ts(h, C4 * (W // 2))
                nc.sync.dma_start(kf[C4:C2, hcols], kf[96:128, hcols])

                # transposes + qk accumulation for this half's 16 chunks
                for bi in range(BPC):
                    Pq = C4 * bi
                    Pk = C2 if bi == 0 else C4
                    for (srcT, dstT, Ps) in ((qf, qfTs[bi], Pq),
                                             (kf, kfTs[bi], Pk)):
                        ps = ps_tr.tile([128, 512], F32, tag="trps")
                        for jj in range(16):
                            j2 = h * 16 + jj
                            nc.tensor.transpose(
                                ps[:, jj * C4:(jj + 1) * C4],
                                srcT[Ps:Ps + C4, j2 * 128:(j2 + 1) * 128],
                                id_f32[Ps:Ps + C4, Ps:Ps + C4])
                        nc.scalar.copy(
                            dstT[:, h * 512:(h + 1) * 512], ps[:])
                    for jj in range(16):
                        j2 = h * 16 + jj
                        nc.tensor.matmul(
                            qkts_ps[bi][:],
                            kfTs[bi][:, j2 * C4:(j2 + 1) * C4],
                            qfTs[bi][:, j2 * C4:(j2 + 1) * C4],
                            start=(h == 0 and jj == 0),
                            stop=(h == 1 and jj == 15))

            # ---------- conv1x1 chunks, x1 strips interleaved ----------
            emit_x1_strip(0)
            emit_x1_strip(1)
            x1_strip_emitted[0] = 2
            rhs = None
            for j in range(NCHUNKS):
                cols = bass.ts(j, NCH)
                if j % 2 == 0:
                    rhs = rhsp.tile([128, 2 * NCH], F32, tag="rhs")
                    cols2 = slice(j * NCH, (j + 2) * NCH)
                    nc.sync.dma_start(rhs[0:C2, :], x2f[0][:, cols2])
                    nc.sync.dma_start(rhs[C2:128, :], x2f[1][:, cols2])
                rhsv = rhs[:, (j % 2) * NCH:(j % 2 + 1) * NCH]

                Ap = ps_conv.tile([128, NCH], F32, tag="convps")
                nc.tensor.matmul(Ap[:], lhsA[:], rhsv, start=True, stop=True)
                qg = rhsp.tile([128, NCH], F32, tag="qg")
                nc.scalar.activation(qg[:], Ap[:], ACTF.Gelu,
                                     bias=biasA[:, 0:1])

                # fused horizontal pooling into rt (both batches per op)
                rrows = rt[:, j * RPC * (W // 2):(j + 1) * RPC * (W // 2)]
                Xq = qg[0:C2, :].rearrange(
                    "p (h w2 two) -> p h w2 two", h=RPC, two=2)
                r3 = rrows[0:C2, :].rearrange("p (h w2) -> p h w2", h=RPC)
                nc.gpsimd.tensor_add(r3[:], Xq[:, :, :, 0], Xq[:, :, :, 1])
                nc.gpsimd.tensor_add(r3[:, :, 1:W // 2], r3[:, :, 1:W // 2],
                                     Xq[:, :, 0:W // 2 - 1, 1])
                Xk = qg[C2:128, :].rearrange(
                    "p (h w2 two) -> p h w2 two", h=RPC, two=2)
                m3 = rrows[C2:128, :].rearrange("p (h w2) -> p h w2", h=RPC)
                nc.vector.tensor_max(m3[:], Xk[:, :, :, 0], Xk[:, :, :, 1])

                Bp = ps_conv.tile([128, NCH], F32, tag="convps")
                nc.tensor.matmul(Bp[:], lhsB[:], rhsv, start=True, stop=True)
                nc.scalar.activation(v_pack[:, cols], Bp[0:C2, :], ACTF.Gelu,
                                     bias=biasB[0:C2, 0:1])
                if j % 2 == 0:
                    lst = lp.tile([128, 2 * NCH], F32, tag="lst")
                nc.scalar.activation(lst[C2:128, (j % 2) * NCH:
                                         (j % 2 + 1) * NCH],
                                     Bp[C2:128, :], ACTF.Gelu,
                                     bias=biasB[C2:128, 0:1])
                if j % 2 == 1:
                    cols2 = slice((j - 1) * NCH, (j + 1) * NCH)
                    nc.sync.dma_start(outf[0][C2:96, cols2], lst[C2:96, :])
                    nc.sync.dma_start(outf[1][C2:96, cols2], lst[96:128, :])

                if j % 6 == 3 and x1_strip_emitted[0] < 7:
                    emit_x1_strip(x1_strip_emitted[0])
                    x1_strip_emitted[0] += 1
                if j == 15 or j == NCHUNKS - 1:
                    emit_attn_half(0 if j == 15 else 1)

            # ---------- attention stats + out2 ----------
            E_tiles = []
            for bi in range(BPC):
                Pv = C4 * bi              # v_pack base: 0 / 32
                qkts = attnp.tile([C4, C4], F32, tag="qkts")
                nc.scalar.mul(qkts[:], qkts_ps[bi][:], 1.0 / 9.0)

                nmax = attnp.tile([C4, 1], F32, tag="nmax")
                nc.vector.tensor_reduce(nmax[:], qkts[:], axis=AX.X,
                                        op=ALU.max, negate=True)
                ET = attnp.tile([C4, C4], F32, tag="ET")
                nc.scalar.activation(ET[:], qkts[:], ACTF.Exp,
                                     bias=nmax[:, 0:1])
                ssum = attnp.tile([C4, 1], F32, tag="ssum")
                nc.vector.reduce_sum(ssum[:], ET[:], axis=AX.X)
                rec = attnp.tile([C4, 1], F32, tag="rec")
                nc.vector.reciprocal(rec[:], ssum[:])
                ETn = attnp.tile([C4, C4], F32, tag="ETn")
                nc.vector.tensor_scalar_mul(ETn[:], ET[:], rec[:, 0:1])

                etp = ps_sm.tile([C4, C4], F32, tag="sm", name=f"etp{bi}")
                nc.tensor.transpose(etp[:], ETn[:], id_f32[0:C4, 0:C4])
                E = attnp.tile([C2, C4], BF16, tag=f"E{bi}")
                nc.scalar.copy(E[0:C4, :], etp[:])
                if Pv != 0:
                    nc.sync.dma_start(E[Pv:Pv + C4, :], E[0:C4, :])
                E_tiles.append(E)

            # out2: pack both batches of a chunk into one psum bank at
            # partition bases {0, 64} -> one evac + 2 DMAs per chunk.
            for j in range(NCHUNKS):
                cols = bass.ts(j, NCH)
                o2 = ps_o2.tile([C2, NCH], F32, tag="o2")
                for bi in range(BPC):
                    Pv = C4 * bi
                    nc.tensor.matmul(
                        o2[C4 * bi:C4 * bi + C4, :],
                        E_tiles[bi][Pv:Pv + C4, :],
                        v_pack[Pv:Pv + C4, cols],
                        start=True, stop=True)
                st = stgp.tile([C2, NCH], F32, tag="st", bufs=4)
                nc.scalar.copy(st[:], o2[:])
                for bi in range(BPC):
                    nc.sync.dma_start(outf[bi][96:128, cols],
                                      st[C4 * bi:C4 * bi + C4, :])

            for s in range(x1_strip_emitted[0], H // R):
                emit_x1_strip(s)

        for _ in range(loops):
            one_pass()

    nc.compile()
    return nc


_NC_CACHE = None


def _get_nc():
    global _NC_CACHE
    if _NC_CACHE is None:
        _NC_CACHE = build_nc()
    return _NC_CACHE


def kernel(x, dw_w, dw_b, qkvl_w, qkvl_b):
    x = np.ascontiguousarray(np.asarray(x, dtype=np.float32))
    shared = {
        "dw_w": np.ascontiguousarray(np.asarray(dw_w, dtype=np.float32)),
        "dw_b": np.ascontiguousarray(np.asarray(dw_b, dtype=np.float32)),
        "qkvl_w": np.ascontiguousarray(np.asarray(qkvl_w, dtype=np.float32)),
        "qkvl_b": np.ascontiguousarray(np.asarray(qkvl_b, dtype=np.float32)),
    }
    nc = _get_nc()
    in_maps = [
        {"x": x[c * BPC:(c + 1) * BPC], **shared} for c in range(N_CORES)
    ]
    res = bass_utils.run_bass_kernel_spmd(nc, in_maps,
                                          core_ids=list(range(N_CORES)))
    return np.concatenate([res.results[c]["out"] for c in range(N_CORES)],
                          axis=0)



# revision 16
# speedup vs baseline: 1.1326x; 1.1326x over previous
"""Trainium2 Bass kernel for the ELGCA block (dwconv3x3+gelu || conv1x1+gelu
-> pooled linear attention), data-parallel over batch on 8 NeuronCores.

Self-contained: hardcodes shapes B=16, C=128, H=W=128, f32.
kernel(**inputs) takes full unsharded inputs, returns the FULL f32 output.

v4 (per core, BPC=2 local images, partitions p = b*64 + c):
  - dwconv3x3 on PE: 9 accumulating matmuls per 512-col chunk with
    diagonal bf16 weights, tap-major across each 16-row window (4
    matmuls per LDWEIGHTS — weight swaps inside accumulation chains
    stall the PE ~160ns, so amortize them), f32 PSUM accumulate.
  - conv1x1: both-batch block-diagonal matmuls; A-side (q|k) in f32
    (softmax logits need it), B-side (v|l) bf16.
  - bulk DMA on the GpSimd SWDGE queue (spreads over all 16 DMA
    engines; the two HWDGE queues share one engine pair), small
    outputs on sync/scalar HWDGE.
  - constants loaded with few descriptors (whole-tensor loads + PE
    transposes + on-chip block builds) — scattered tiny-descriptor
    DMAs serialize the queues for tens of us.
  - all outputs written bf16, widened to f32 on the host.
"""

import numpy as np
from contextlib import ExitStack

import concourse.bass as bass
import concourse.tile as tile
from concourse import bacc, mybir
from concourse import bass_utils
from concourse.masks import make_identity

F32 = mybir.dt.float32
BF16 = mybir.dt.bfloat16
AX = mybir.AxisListType
ALU = mybir.AluOpType
ACTF = mybir.ActivationFunctionType

N_CORES = 8
B_TOT, C, H, W = 16, 128, 128, 128
BPC = B_TOT // N_CORES          # 2 images per core
HW = H * W                      # 16384
C2 = C // 2                     # 64
C4 = C // 4                     # 32
WP = W + 2                      # padded row width (130)
NW = 8                          # number of 16-row windows
WR = H // NW                    # image rows per window (16)
NP = (H // 2) * (W // 2)        # 4096 pooled positions
W2 = W // 2                     # 64


def build_nc(loops=1):
    nc = bacc.Bacc("TRN2", target_bir_lowering=False, debug=False,
                   num_devices=N_CORES)
    x = nc.dram_tensor("x", [BPC, C, H, W], F32, kind="ExternalInput").ap()
    dw_w = nc.dram_tensor("dw_w", [C2, 1, 3, 3], F32, kind="ExternalInput").ap()
    dw_b = nc.dram_tensor("dw_b", [C2], F32, kind="ExternalInput").ap()
    qw = nc.dram_tensor("qkvl_w", [C, C2, 1, 1], F32, kind="ExternalInput").ap()
    qb = nc.dram_tensor("qkvl_b", [C], F32, kind="ExternalInput").ap()
    out = nc.dram_tensor("out", [BPC, C, H, W], BF16, kind="ExternalOutput").ap()

    with tile.TileContext(nc) as tc, ExitStack() as ctx:
        consts = ctx.enter_context(tc.tile_pool(name="consts", bufs=1))
        inp = ctx.enter_context(tc.tile_pool(name="inp", bufs=2))
        slabp = ctx.enter_context(tc.tile_pool(name="slabp", bufs=2))
        bigp = ctx.enter_context(tc.tile_pool(name="bigp", bufs=1))
        stgp = ctx.enter_context(tc.tile_pool(name="stgp", bufs=2))
        ps = ctx.enter_context(tc.tile_pool(name="ps", bufs=1, space="PSUM"))

        # ---------------- constants (few-descriptor loads) ----------------
        id_f32 = consts.tile([128, 128], F32)
        make_identity(nc, id_f32[:])

        # dw weights: [64, 9] rows -> dup to [128, 9]
        w_tile = consts.tile([128, 9], F32)
        dw9 = dw_w.rearrange("c o kh kw -> c (o kh kw)")
        nc.gpsimd.dma_start(w_tile[0:C2, :], dw9)
        nc.gpsimd.dma_start(w_tile[C2:128, :], dw9)

        # qkvl_w: load [128oc, 64ic] contiguous, PE-transpose to [64ic, 128oc]
        qw_oc = consts.tile([128, C2], F32)
        nc.gpsimd.dma_start(qw_oc[:], qw.rearrange("o i kh kw -> o (i kh kw)"))
        qwT_ps = ps.tile([128, 1024], F32, tag="cv", bufs=2)
        nc.tensor.transpose(qwT_ps[0:C2, 0:128], qw_oc[:], id_f32[:])
        qwT_sb = consts.tile([C2, 128], F32)
        nc.scalar.copy(qwT_sb[:], qwT_ps[0:C2, 0:128])

        # row-vector loads (1 descriptor each) for biases
        qb_row = consts.tile([1, C], F32)
        nc.gpsimd.dma_start(qb_row[:], qb.unsqueeze(0))
        dwb_row = consts.tile([1, C2], F32)
        nc.gpsimd.dma_start(dwb_row[:], dw_b.unsqueeze(0))

        # permuted bias rows -> PE transpose to per-partition columns
        # brow col-blocks: biasA = [qb0:32|qb0:32|qb32:64|qb32:64],
        # biasB = [qb64:96|...], dwb = [dwb|dwb]
        brow = consts.tile([1, 3 * 128], F32)
        nc.scalar.copy(brow[:, 0:C4], qb_row[:, 0:C4])
        nc.scalar.copy(brow[:, C4:C2], qb_row[:, 0:C4])
        nc.scalar.copy(brow[:, C2:96], qb_row[:, C4:C2])
        nc.scalar.copy(brow[:, 96:128], qb_row[:, C4:C2])
        nc.scalar.copy(brow[:, 128:160], qb_row[:, C2:96])
        nc.scalar.copy(brow[:, 160:192], qb_row[:, C2:96])
        nc.scalar.copy(brow[:, 192:224], qb_row[:, 96:128])
        nc.scalar.copy(brow[:, 224:256], qb_row[:, 96:128])
        nc.scalar.copy(brow[:, 256:320], dwb_row[:, 0:C2])
        nc.scalar.copy(brow[:, 320:384], dwb_row[:, 0:C2])
        bcol_ps = ps.tile([128, 1024], F32, tag="dwA")
        for i in range(3):
            nc.tensor.transpose(bcol_ps[:, i:i + 1],
                                brow[0:1, i * 128:(i + 1) * 128],
                                id_f32[0:1, 0:1])
        bcol = consts.tile([128, 3], F32)
        nc.scalar.copy(bcol[:], bcol_ps[:, 0:3])
        biasA = bcol[:, 0:1]
        biasB = bcol[:, 1:2]
        dwb_t = bcol[:, 2:3]

        # 9 diagonal tap matrices, bf16
        wdiag_f = consts.tile([128, 9 * 128], F32)
        wdiag = consts.tile([128, 9 * 128], BF16)
        for t in range(9):
            nc.vector.tensor_scalar_mul(
                wdiag_f[:, t * 128:(t + 1) * 128], id_f32[:],
                w_tile[:, t:t + 1])
        nc.vector.tensor_copy(wdiag[:], wdiag_f[:])

        # conv1x1 block-diagonal weights from qwT_sb (on-chip copies)
        lhs_f = consts.tile([128, 256], F32)
        nc.vector.memset(lhs_f[:], 0.0)
        nc.scalar.copy(lhs_f[0:C2, 0:C4], qwT_sb[:, 0:C4])
        nc.scalar.copy(lhs_f[C2:128, C4:C2], qwT_sb[:, 0:C4])
        nc.scalar.copy(lhs_f[0:C2, C2:96], qwT_sb[:, C4:C2])
        nc.scalar.copy(lhs_f[C2:128, 96:128], qwT_sb[:, C4:C2])
        nc.scalar.copy(lhs_f[0:C2, 128:160], qwT_sb[:, C2:96])
        nc.scalar.copy(lhs_f[C2:128, 160:192], qwT_sb[:, C2:96])
        nc.scalar.copy(lhs_f[0:C2, 192:224], qwT_sb[:, 96:128])
        nc.scalar.copy(lhs_f[C2:128, 224:256], qwT_sb[:, 96:128])
        lhsAB = consts.tile([128, 256], BF16)
        nc.vector.tensor_copy(lhsAB[:], lhs_f[:])
        lhsA = lhs_f[:, 0:128]          # f32: qk logits need full precision
        lhsB = lhsAB[:, 128:256]

        def one_pass():
            # persistent per-pass buffers
            vl = bigp.tile([128, HW], BF16, tag="vl")   # v rows 0:64, l 64:128
            hp = bigp.tile([128, H * W2], F32, tag="hp")
            hp3 = hp.rearrange("p (r w) -> p r w", w=W2)
            hpv = hp.rearrange("p (o two w) -> p o two w", two=2, w=W2)
            pp = bigp.tile([128, NP], F32, tag="pp")
            pp3 = pp.rearrange("p (r w) -> p r w", w=W2)
            trsb = bigp.tile([128, 2048], F32, tag="trsb")
            qk_acc = bigp.tile([C2, C2], F32, tag="qk")

            for w in range(NW):
                y0 = w * WR
                ys = max(y0 - 1, 0)
                ye = min(y0 + WR + 1, H)
                nrows = ye - ys
                rs = 0 if w > 0 else 1

                # ---- input DMA + casts ----
                xtmp = inp.tile([128, 18 * W], F32, tag="xtmp")
                xtmp3 = xtmp.rearrange("p (r w) -> p r w", w=W)
                nc.gpsimd.dma_start(xtmp3[:, 0:nrows, :],
                                    x[0:BPC, 0:C2, ys:ye, :])
                x2tmp = inp.tile([128, WR * W], F32, tag="x2tmp")
                nc.gpsimd.dma_start(
                    x2tmp.rearrange("p (r w) -> p r w", w=W)[:],
                    x[0:BPC, C2:C, y0:y0 + WR, :])

                slab = slabp.tile([128, 18 * WP], BF16, tag="slab")
                slab3 = slab.rearrange("p (r w) -> p r w", w=WP)
                nc.gpsimd.memset(slab3[:, :, 0:1], 0.0)
                nc.gpsimd.memset(slab3[:, :, WP - 1:WP], 0.0)
                if w == 0:
                    nc.gpsimd.memset(slab3[:, 0:1, :], 0.0)
                if w == NW - 1:
                    nc.gpsimd.memset(slab3[:, 17:18, :], 0.0)
                nc.vector.tensor_copy(slab3[:, rs:rs + nrows, 1:W + 1],
                                      xtmp3[:, 0:nrows, :])
                x2bf = inp.tile([128, WR * W], BF16, tag="x2bf")
                nc.vector.tensor_copy(x2bf[:], x2tmp[:])

                qg = stgp.tile([128, WR * W], F32, tag="qg")
                x1st = stgp.tile([128, WR * W], BF16, tag="x1st")

                # ---- conv1x1 A (f32) and B (bf16), 1024-col pairs ----
                for pr in range(2):
                    pc0 = pr * 1024
                    Aps = ps.tile([128, 1024], F32, tag="cv", bufs=2)
                    for hf in range(2):
                        nc.tensor.matmul(
                            Aps[:, hf * 512:(hf + 1) * 512], lhsA,
                            x2tmp[:, pc0 + hf * 512:pc0 + (hf + 1) * 512],
                            start=True, stop=True)
                    nc.scalar.activation(qg[:, pc0:pc0 + 1024], Aps[:],
                                         ACTF.Gelu, bias=biasA)
                    Bps = ps.tile([128, 1024], F32, tag="cv", bufs=2)
                    for hf in range(2):
                        nc.tensor.matmul(
                            Bps[:, hf * 512:(hf + 1) * 512], lhsB,
                            x2bf[:, pc0 + hf * 512:pc0 + (hf + 1) * 512],
                            start=True, stop=True)
                    nc.scalar.activation(vl[:, y0 * W + pc0:
                                            y0 * W + pc0 + 1024],
                                         Bps[:], ACTF.Gelu, bias=biasB)

                # ---- dwconv: tap-major across the whole window ----
                dwA = ps.tile([128, 1024], F32, tag="dwA")
                dwB = ps.tile([128, 1024], F32, tag="dwB")
                for t in range(9):
                    dy, dx = t // 3, t % 3
                    for q in range(4):
                        tgt = dwA if q < 2 else dwB
                        la = q * 4
                        nc.tensor.matmul(
                            tgt[:, (q % 2) * 512:(q % 2 + 1) * 512],
                            wdiag[:, t * 128:(t + 1) * 128],
                            slab3[:, la + dy:la + dy + 4, dx:dx + W],
                            start=(t == 0), stop=(t == 8))
                nc.scalar.activation(x1st[:, 0:1024], dwA[:], ACTF.Gelu,
                                     bias=dwb_t)
                nc.scalar.activation(x1st[:, 1024:2048], dwB[:], ACTF.Gelu,
                                     bias=dwb_t)

                # ---- output DMA for this window ----
                nc.gpsimd.dma_start(
                    out[0:BPC, 0:C2, y0:y0 + WR, :],
                    x1st.rearrange("p (r w) -> p r w", w=W)[:])
                nc.sync.dma_start(
                    out[0:BPC, C2:96, y0:y0 + WR, :],
                    vl[C2:128, y0 * W:(y0 + WR) * W]
                    .rearrange("p (r w) -> p r w", w=W))

                # ---- horizontal pooling for this window ----
                qg3 = qg.rearrange("p (r w2 two) -> p r w2 two", two=2, w2=W2)
                nc.gpsimd.tensor_add(hp3[0:C2, y0:y0 + WR, :],
                                     qg3[0:C2, :, :, 0], qg3[0:C2, :, :, 1])
                nc.gpsimd.tensor_add(hp3[0:C2, y0:y0 + WR, 1:W2],
                                     hp3[0:C2, y0:y0 + WR, 1:W2],
                                     qg3[0:C2, :, 0:W2 - 1, 1])
                nc.vector.tensor_max(hp3[C2:128, y0:y0 + WR, :],
                                     qg3[C2:128, :, :, 0],
                                     qg3[C2:128, :, :, 1])

                # ---- per-half: v-pool + transposes + qk ----
                if w == NW // 2 - 1 or w == NW - 1:
                    h = 0 if w == NW // 2 - 1 else 1
                    o0 = h * C4
                    # q v-pool: taps rows 2oy-1, 2oy, 2oy+1 (top pad 0)
                    nc.vector.tensor_add(pp3[0:C2, o0:o0 + C4, :],
                                         hpv[0:C2, o0:o0 + C4, 0, :],
                                         hpv[0:C2, o0:o0 + C4, 1, :])
                    lo = max(o0, 1)
                    nc.vector.tensor_add(pp3[0:C2, lo:o0 + C4, :],
                                         pp3[0:C2, lo:o0 + C4, :],
                                         hpv[0:C2, lo - 1:o0 + C4 - 1, 1, :])
                    # k v-max of row pairs
                    nc.vector.tensor_max(pp3[C2:128, o0:o0 + C4, :],
                                         hpv[C2:128, o0:o0 + C4, 0, :],
                                         hpv[C2:128, o0:o0 + C4, 1, :])
                    # transposes: 16 chunks of 128 pooled positions
                    for g in range(2):
                        trps = ps.tile([128, 1024], F32,
                                       tag=("dwA" if g == 0 else "dwB"))
                        for jj in range(8):
                            ch = h * 16 + g * 8 + jj
                            nc.tensor.transpose(
                                trps[:, jj * 128:(jj + 1) * 128],
                                pp[:, ch * 128:(ch + 1) * 128],
                                id_f32[:])
                        nc.scalar.copy(trsb[:, g * 1024:(g + 1) * 1024],
                                       trps[:])
                    qkps = ps.tile([128, 1024], F32, tag="dwA")
                    for jj in range(16):
                        nc.tensor.matmul(
                            qkps[0:C2, 0:C2],
                            trsb[:, jj * 128 + C2:(jj + 1) * 128],
                            trsb[:, jj * 128:jj * 128 + C2],
                            start=(jj == 0), stop=(jj == 15))
                    if h == 0:
                        nc.scalar.copy(qk_acc[:], qkps[0:C2, 0:C2])
                    else:
                        nc.vector.tensor_add(qk_acc[:], qk_acc[:],
                                             qkps[0:C2, 0:C2])

            # ---------- softmax stats -> block-diag attention ----------
            Ebd = bigp.tile([C2, C2], BF16, tag="Ebd")
            nc.vector.memset(Ebd[:], 0.0)
            qk9 = bigp.tile([C2, C2], F32, tag="qk9")
            nc.scalar.mul(qk9[:], qk_acc[:], 1.0 / 9.0)
            for bi in range(BPC):
                o = C4 * bi
                blk = qk9[o:o + C4, o:o + C4]
                nmax = bigp.tile([C4, 1], F32, tag=f"nmax{bi}")
                nc.vector.tensor_reduce(nmax[:], blk, axis=AX.X,
                                        op=ALU.max, negate=True)
                ET = bigp.tile([C4, C4], F32, tag=f"ET{bi}")
                nc.scalar.activation(ET[:], blk, ACTF.Exp,
                                     bias=nmax[:, 0:1])
                ssum = bigp.tile([C4, 1], F32, tag=f"ssum{bi}")
                nc.vector.reduce_sum(ssum[:], ET[:], axis=AX.X)
                rec = bigp.tile([C4, 1], F32, tag=f"rec{bi}")
                nc.vector.reciprocal(rec[:], ssum[:])
                ETn = bigp.tile([C4, C4], F32, tag=f"ETn{bi}")
                nc.vector.tensor_scalar_mul(ETn[:], ET[:], rec[:, 0:1])
                etp = ps.tile([128, 1024], F32, tag="dwB")
                nc.tensor.transpose(etp[0:C4, 0:C4], ETn[:],
                                    id_f32[0:C4, 0:C4])
                nc.scalar.copy(Ebd[o:o + C4, o:o + C4], etp[0:C4, 0:C4])

            # ---------- out2 = attn @ v, both batches per matmul ----------
            for w in range(NW):
                y0 = w * WR
                o2st = stgp.tile([C2, WR * W], BF16, tag="o2st")
                for pr in range(2):
                    pc0 = pr * 1024
                    o2ps = ps.tile([128, 1024], F32, tag="cv", bufs=2)
                    for hf in range(2):
                        nc.tensor.matmul(
                            o2ps[0:C2, hf * 512:(hf + 1) * 512], Ebd[:],
                            vl[0:C2, y0 * W + pc0 + hf * 512:
                               y0 * W + pc0 + (hf + 1) * 512],
                            start=True, stop=True)
                    nc.vector.tensor_copy(o2st[:, pc0:pc0 + 1024],
                                          o2ps[0:C2, :])
                nc.scalar.dma_start(
                    out[0:BPC, 96:128, y0:y0 + WR, :],
                    o2st.rearrange("p (r w) -> p r w", w=W)[:])

        for _ in range(loops):
            one_pass()

    nc.compile()
    return nc


_NC_CACHE = None


def _get_nc():
    global _NC_CACHE
    if _NC_CACHE is None:
        _NC_CACHE = build_nc()
    return _NC_CACHE


def kernel(x, dw_w, dw_b, qkvl_w, qkvl_b):
    x = np.ascontiguousarray(np.asarray(x, dtype=np.float32))
    shared = {
        "dw_w": np.ascontiguousarray(np.asarray(dw_w, dtype=np.float32)),
        "dw_b": np.ascontiguousarray(np.asarray(dw_b, dtype=np.float32)),
        "qkvl_w": np.ascontiguousarray(np.asarray(qkvl_w, dtype=np.float32)),
        "qkvl_b": np.ascontiguousarray(np.asarray(qkvl_b, dtype=np.float32)),
    }
    nc = _get_nc()
    in_maps = [
        {"x": x[c * BPC:(c + 1) * BPC], **shared} for c in range(N_CORES)
    ]
    res = bass_utils.run_bass_kernel_spmd(nc, in_maps,
                                          core_ids=list(range(N_CORES)))
    return np.concatenate(
        [np.asarray(res.results[c]["out"]).astype(np.float32)
         for c in range(N_CORES)], axis=0)
